# revision 14
# baseline (speedup 1.0000x reference)
"""NT-Xent loss kernel for Trainium2 (8 NeuronCores, one SPMD program).

Reference (N=4096, D=256, T=0.5):
    zn = l2norm(rows of [z_i; z_j]); sim = zn @ zn.T
    lse_a = ln sum_{b!=a} exp(2 sim_ab);  pos_a = sim[a, a+-N]
    loss = mean(lse_a - 2 pos_a)

Triangle sharding, core-uniform: on the 64x64 grid of 128x128 sim tiles,
each unordered tile pair is computed once:
  * super-diagonal 8x8-tile block I: internal upper triangle, computed by
    core I against its own row tiles ("zrows" slots 0-7, rhs = itself) --
    no per-core addresses.
  * suffix strips: row tile 8*si + c (core c, "zrows" slot 8+si) x columns
    [8*(si+1), 64): the column START is uniform; core identity lives only
    in the input data.  260 exp-tiles per core, perfectly balanced.
Row sums come from ACTIVATE-exp accumulators; column sums (for the mirrored
lower triangle) accumulate in bf16 E_acc via DVE/GpSimd adds and are
finished on the host together with the final ln/mean ("all-reduce").

Engine plan: transposes go through the DMA XBAR (bf16, 2-byte path), main
matmuls are fp8e4 DoubleRow (K=256 in one pass), exp runs in 2048-wide
ACTIVATEs on ScalarE (the bound), element-wise prep splits DVE/GpSimd.
"""

import sys

for _p in ("/opt/trn_rl_repo",):
    if _p not in sys.path:
        sys.path.insert(0, _p)

import numpy as np
from contextlib import ExitStack

import concourse.bass as bass
import concourse.tile as tile
from concourse import mybir
from concourse.vector_clock import ScopedClock as _ScopedClock


def _patched_drain_and_barrier(self, tick_clock, wait_clock):
    """Tile's closing drain carries one sem-wait per DMA lane used, but this
    walrus build only accepts a single sync wait on a Drain (CTRL-NO)
    lowering ("Too many sync wait commands").  Split the waits across a
    chain of drains (sequential on SP, so semantics are unchanged)."""
    nc = self.nc
    drain_inst = nc.sync.drain()
    wait_clock.add_sem_waits(
        drain_inst.ins, _ScopedClock({None: tick_clock.global_clock})
    )
    si = drain_inst.ins.sync_info
    if si is not None:
        waits = list(si.on_wait or [])
        if len(waits) > 1:
            import bass_rust as _br

            si.on_wait = waits[:1]
            for w in waits[1:]:
                d2 = nc.sync.drain()
                d2.ins.sync_info = _br.SyncInfo(on_wait=[w], on_update=[])
    nc.all_engine_barrier()
    assert self.sems is not None
    popped = nc._tile_sem_poison_stack.pop()
    assert popped is self._sem_poison
    nc.clear_and_free_semaphores(list(self.sems.allocated().values()))
    nc.all_engine_barrier()


tile.TileContext._drain_and_barrier = _patched_drain_and_barrier

_orig_lower_ordered = tile.TileContext._lower_ordered_insts


def _split_multiwaits_and_lower(self, ordered):
    """Same walrus limitation as above, for scheduled compute/DMA
    instructions: hoist all but one sync wait onto single-wait NoOps that
    precede the instruction on its own engine."""
    nc = self.nc
    for insts in ordered.values():
        if not any(
            inst.sync_info is not None and len(inst.sync_info.on_wait or []) > 1
            for inst in insts
        ):
            continue
        out = []
        for inst in insts:
            si = inst.sync_info
            waits = list(si.on_wait) if si is not None and si.on_wait else []
            if len(waits) > 1 and getattr(inst, "engine", None) is not None:
                for w in waits[:-1]:
                    out.append(
                        mybir.InstNoOp(
                            name=nc.get_next_instruction_name(),
                            sync_info=mybir.SyncInfo(on_wait=[w], on_update=[]),
                            bass_nofuse=True,
                            engine=inst.engine,
                        )
                    )
                si.on_wait = waits[-1:]
            out.append(inst)
        insts[:] = out
    return _orig_lower_ordered(self, ordered)


tile.TileContext._lower_ordered_insts = _split_multiwaits_and_lower

N_CORES = 8
N_FULL = 4096
D_FULL = 256

f32 = mybir.dt.float32
f16 = mybir.dt.float16
bf16 = mybir.dt.bfloat16
fp8 = mybir.dt.float8e4
ALU = mybir.AluOpType
AF = mybir.ActivationFunctionType
DR = mybir.MatmulPerfMode.DoubleRow

# descending 16-tile chunks of the global column range [8, 64)
CHUNKS = [(48, 16), (32, 16), (16, 16), (8, 8)]


def strip_cols(si):
    """Suffix strip si covers global column tiles [8*(si+1), 64)."""
    return 8 * (si + 1)


def build_bass(N=N_FULL, D=D_FULL, n_cores=N_CORES):
    n2 = 2 * N
    TF = n2 // 128            # 64 global tiles
    KH = D // 128             # 2 contraction halves
    NR = 16                   # zrows tiles: 8 super-diag + 8 strip rows
    assert D == 256 and TF == 64

    nc = bass.Bass()
    z_i = nc.declare_dram_parameter("z_i", [N, D], f32, isOutput=False)
    z_j = nc.declare_dram_parameter("z_j", [N, D], f32, isOutput=False)
    zr = nc.declare_dram_parameter("zrows", [NR * 128, D], f32, isOutput=False)
    eacc_out = nc.declare_dram_parameter("eacc", [128, n2], bf16, isOutput=True)
    eint_out = nc.declare_dram_parameter("eint", [128, 1024], bf16, isOutput=True)
    sp_out = nc.declare_dram_parameter("spart", [128, 36], f32, isOutput=True)
    pos_out = nc.declare_dram_parameter("posr", [128, 8], f32, isOutput=True)

    with ExitStack() as ctx:
        tc = ctx.enter_context(tile.TileContext(nc))
        big = ctx.enter_context(tc.tile_pool(name="big", bufs=1))
        zfs = ctx.enter_context(tc.tile_pool(name="zfs", bufs=2))
        zns = ctx.enter_context(tc.tile_pool(name="zns", bufs=2))
        zts = ctx.enter_context(tc.tile_pool(name="zts", bufs=2))
        sqs = ctx.enter_context(tc.tile_pool(name="sqs", bufs=2))
        escr = ctx.enter_context(tc.tile_pool(name="escr", bufs=3))
        pmm = ctx.enter_context(tc.tile_pool(name="pmm", bufs=2, space="PSUM"))

        znT8 = big.tile([128, KH, n2], fp8)       # global transposed fp8
        ssq = big.tile([128, TF], f16)
        lnss = big.tile([128, TF], f32)
        invn = big.tile([128, TF], f32)

        znr = big.tile([128, KH, NR, 128], bf16)  # zrows normalized (kept: pos)
        zrT8 = big.tile([128, KH, NR * 128], fp8)
        ssqr = big.tile([128, NR], f16)
        lnssr = big.tile([128, NR], f32)
        invnr = big.tile([128, NR], f32)

        E_acc = big.tile([128, n2], bf16)         # strip colsum partials
        E_int = big.tile([128, 1024], bf16)       # super-diag colsum partials
        Spart = big.tile([128, 36], f32)          # rowsum accum slots
        posr = big.tile([128, 8], f32)

        def prep(nt, src_ap, dst_zn, dst_T8, d_ssq, d_lnss, d_invn, qoff,
                 t8off, gp_frac):
            """load -> sumsq -> invn -> normalize -> XBAR -> fp8 cast for
            nt tiles.  dst_zn: [128, KH, nt, 128] region; dst_T8 columns
            start at t8off*128.  Returns nothing; emits instructions."""
            zf = zfs.tile([128, 16, D], bf16, tag="zf")
            nc.gpsimd.dma_start(out=zf[:, :nt, :], in_=src_ap)
            sq = sqs.tile([128, 16, D], bf16, tag="sq")
            q = slice(qoff, qoff + nt)
            nc.vector.tensor_mul(out=sq[:, :nt, :], in0=zf[:, :nt, :],
                                 in1=zf[:, :nt, :])
            with nc.allow_low_precision("fp16 sumsq: |z|^2~256, rel 5e-4"):
                nc.vector.reduce_sum(out=d_ssq[:, q], in_=sq[:, :nt, :],
                                     axis=mybir.AxisListType.X)
            nc.scalar.activation(out=d_lnss[:, q], in_=d_ssq[:, q], func=AF.Ln)
            nc.scalar.activation(out=d_invn[:, q], in_=d_lnss[:, q],
                                 func=AF.Exp, scale=-0.5)
            zn = dst_zn if dst_zn is not None else zns.tile(
                [128, KH, 16, 128], bf16, tag="zn")
            n_gp = int(nt * gp_frac)
            for k in range(nt):
                eng = nc.gpsimd if k < n_gp else nc.vector
                eng.tensor_scalar_mul(
                    out=zn[:, :, k, :],
                    in0=zf[:, k, :].rearrange("p (h d) -> p h d", h=KH),
                    scalar1=d_invn[:, qoff + k : qoff + k + 1],
                )
            zT = zts.tile([128, KH, 16, 128], bf16, tag="zT")
            for h in range(KH):
                nc.sync.dma_start(
                    out=zT[:, h, :nt, :],
                    in_=zn[:, h, 0:nt, :].rearrange("p t d -> p (t d)"),
                    transpose=True,
                )
            for h in range(KH):
                eng = nc.gpsimd if h == 0 else nc.vector
                eng.tensor_copy(
                    out=dst_T8[:, h, t8off * 128 : (t8off + nt) * 128],
                    in_=zT[:, h, 0:nt, :].rearrange("p t d -> p (t d)"),
                )

        def sim_exp_group(lhs_slot_T8, lhs_col, t0, w, rhs_T8, rhs_t0,
                          sp_slot, e_dst_acc, acc_t0, first_touch):
            """One sim block row: lhsT column lhs_col (128 rows), columns
            tiles [t0, t0+w) of rhs_T8 (tile coords rel. rhs_t0), exp with
            rowsum accum into Spart[:, sp_slot]; colsum pieces into
            e_dst_acc starting at tile acc_t0 (None = skip colsum; int =
            first colsum tile, e-columns before it skipped)."""
            cols = w * 128
            ps = pmm.tile([128, 2048], f32, tag="ps")
            for j0 in range(0, w, 4):
                wj = min(4, w - j0) * 128
                c0 = (t0 - rhs_t0 + j0) * 128
                nc.tensor.matmul(
                    out=ps[:, j0 * 128 : j0 * 128 + wj],
                    lhsT=lhs_slot_T8[:, :, lhs_col * 128 : (lhs_col + 1) * 128],
                    rhs=rhs_T8[:, :, c0 : c0 + wj],
                    start=True, stop=True, perf_mode=DR,
                )
            e = escr.tile([128, 2048], bf16, tag="e")
            nc.scalar.activation(
                out=e[:, :cols], in_=ps[:, :cols], func=AF.Exp, scale=2.0,
                accum_out=Spart[:, sp_slot : sp_slot + 1],
            )
            if acc_t0 is None:
                return
            a = acc_t0          # first colsum tile (>= t0)
            b = t0 + w
            if a >= b:
                return
            eng = nc.vector if (sp_slot % 2 == 0) else nc.gpsimd
            d = e_dst_acc[:, a * 128 : b * 128]
            s = e[:, (a - t0) * 128 : (b - t0) * 128]
            if first_touch:
                eng.tensor_copy(out=d, in_=s)
            else:
                eng.tensor_tensor(out=d, in0=d, in1=s, op=ALU.add)

        # ---- zrows first: slots 0-7 = super-diag rows, 8-15 = strip rows
        prep(16, zr[:, :].rearrange("(t p) d -> p t d", p=128),
             znr, zrT8, ssqr, lnssr, invnr, 0, 0, 0.5)

        # ---- super-diagonal block: internal upper triangle over slots 0-7
        for a in range(8):
            w = 8 - a
            sim_exp_group(zrT8, a, a, w, zrT8, 0, 28 + a,
                          E_int, a + 1, first_touch=(a == 0))

        # ---- stream global chunks high->low; emit ready strip groups ----
        zi_r = z_i[:, :].rearrange("(t p) d -> p t d", p=128)
        zj_r = z_j[:, :].rearrange("(t p) d -> p t d", p=128)
        for (c0, nt) in CHUNKS:
            src = (zj_r[:, c0 - 32 : c0 - 32 + nt, :] if c0 >= 32
                   else zi_r[:, c0 : c0 + nt, :])
            prep(nt, src, None, znT8, ssq, lnss, invn, c0, c0, 0.3)
            for si in range(7):
                cs = strip_cols(si)          # 8*(si+1)
                t0 = max(cs, c0)
                t1 = c0 + nt
                if t0 >= t1:
                    continue
                g = t0 // 16                 # group ordinal by 16-grid
                sp_slot = si * 4 + g
                sim_exp_group(zrT8, 8 + si, t0, t1 - t0, znT8, 0, sp_slot,
                              E_acc, t0, first_touch=(si == 0))

        # ---- positive pairs: strip slots si=0..3 vs si+4 ----
        pmul = big.tile([128, KH, 4, 128], bf16)
        nc.vector.tensor_mul(out=pmul[:, :, :, :], in0=znr[:, :, 8:12, :],
                             in1=znr[:, :, 12:16, :])
        nc.vector.reduce_sum(
            out=posr[:, :].rearrange("p (h s) -> p h s", h=KH),
            in_=pmul[:, :, :, :], axis=mybir.AxisListType.X)

        nc.sync.dma_start(out=eacc_out[:, :], in_=E_acc)
        nc.sync.dma_start(out=eint_out[:, :], in_=E_int)
        nc.sync.dma_start(out=sp_out[:, :], in_=Spart)
        nc.sync.dma_start(out=pos_out[:, :], in_=posr)

    return nc


_NC_CACHE = {}


def _get_nc(N=N_FULL, D=D_FULL):
    key = (N, D)
    if key not in _NC_CACHE:
        _NC_CACHE[key] = build_bass(N, D)
    return _NC_CACHE[key]


def make_in_maps(z_i, z_j, n_cores=N_CORES):
    z_i = np.ascontiguousarray(z_i, dtype=np.float32)
    z_j = np.ascontiguousarray(z_j, dtype=np.float32)
    reps = np.concatenate([z_i, z_j], axis=0)
    maps = []
    for c in range(n_cores):
        rows = [reps[(8 * c + a) * 128 : (8 * c + a + 1) * 128] for a in range(8)]
        rows += [reps[(8 * si + c) * 128 : (8 * si + c + 1) * 128]
                 for si in range(8)]
        maps.append({
            "z_i": z_i,
            "z_j": z_j,
            "zrows": np.ascontiguousarray(np.concatenate(rows, axis=0)),
        })
    return maps


def assemble(results, N=N_FULL):
    """Host-side gather + reduction + final ln/mean ("all-reduce")."""
    n2 = 2 * N
    S = np.zeros(n2, dtype=np.float64)
    pos = np.zeros(N, dtype=np.float64)
    for c in range(N_CORES):
        r0 = results[c]
        sp = np.asarray(r0["spart"], dtype=np.float64)     # [128, 36]
        ea = np.asarray(r0["eacc"], dtype=np.float32)
        ei = np.asarray(r0["eint"], dtype=np.float32)
        # super-diag rowsums: slot 28+a -> rows (8c+a)*128+p
        for a in range(8):
            S[(8 * c + a) * 128 : (8 * c + a + 1) * 128] += sp[:, 28 + a]
        # strip rowsums: slots si*4+g -> rows (8si+c)*128+p; only the slots
        # the kernel actually wrote (mirror its chunk loop)
        for si in range(7):
            r = 8 * si + c
            acc = np.zeros(128, dtype=np.float64)
            for (c0, nt) in CHUNKS:
                t0 = max(strip_cols(si), c0)
                if t0 < c0 + nt:
                    acc += sp[:, si * 4 + t0 // 16]
            S[r * 128 : (r + 1) * 128] += acc
        # super-diag colsums: E_int col b*128+q -> row (8c+b)*128+q
        # (local tile 0 has no internal colsum contributions -> unwritten)
        eint_sum = ei[:, 128:].astype(np.float64).sum(axis=0)  # [896]
        S[8 * c * 128 + 128 : 8 * c * 128 + 1024] += eint_sum
        # strip colsums: global columns >= 1024
        S[1024:] += ea[:, 1024:].astype(np.float64).sum(axis=0)
        # pos: strip slot si covers tile 8si+c (si=0..3); two K-halves
        # laid out h-major in posr [128, (h, si)]
        pr = np.asarray(r0["posr"], dtype=np.float64)
        for si in range(4):
            t = 8 * si + c
            pos[t * 128 : (t + 1) * 128] = pr[:, si] + pr[:, si + 4]
    S -= np.exp(2.0)  # self-similarity exp(2*|zn_a|^2)
    lse = np.log(S)
    loss = np.mean(lse - 2.0 * np.concatenate([pos, pos]))
    return np.float32(loss)


def _run(z_i, z_j, trace=False, tmpdir=None, **spmd_kwargs):
    from concourse.bass_utils import run_bass_kernel_spmd

    N, D = z_i.shape
    nc = _get_nc(N, D)
    in_maps = make_in_maps(z_i, z_j)
    out = run_bass_kernel_spmd(
        nc, in_maps, list(range(N_CORES)), trace=trace, tmpdir=tmpdir,
        **spmd_kwargs
    )
    return assemble(out.results, N), out


def kernel(z_i, z_j):
    loss, _ = _run(np.asarray(z_i), np.asarray(z_j))
    return loss


if __name__ == "__main__":
    rng = np.random.default_rng(0)
    z_i = rng.standard_normal((N_FULL, D_FULL), dtype=np.float32)
    z_j = rng.standard_normal((N_FULL, D_FULL), dtype=np.float32)
    print(kernel(z_i, z_j))


# revision 21
# speedup vs baseline: 1.3087x; 1.3087x over previous
"""NT-Xent loss kernel for Trainium2 (8 NeuronCores, one SPMD program).

Reference (N=4096, D=256, T=0.5):
    zn = l2norm(rows of [z_i; z_j]); sim = zn @ zn.T
    lse_a = ln sum_{b!=a} exp(2 sim_ab);  pos_a = sim[a, a+-N]
    loss = mean(lse_a - 2 pos_a)

Triangle sharding, core-uniform: on the 64x64 grid of 128x128 sim tiles,
each unordered tile pair is computed once:
  * super-diagonal 8x8-tile block I: internal upper triangle, computed by
    core I against its own row tiles ("zrows" slots 0-7, rhs = itself) --
    no per-core addresses.
  * suffix strips: row tile 8*si + c (core c, "zrows" slot 8+si) x columns
    [8*(si+1), 64): the column START is uniform; core identity lives only
    in the input data.  260 exp-tiles per core, perfectly balanced.
Row sums come from ACTIVATE-exp accumulators; column sums (for the mirrored
lower triangle) accumulate in bf16 E_acc via DVE/GpSimd adds and are
finished on the host together with the final ln/mean ("all-reduce").

Engine plan: transposes go through the DMA XBAR (bf16, 2-byte path), main
matmuls are fp8e4 DoubleRow (K=256 in one pass), exp runs in 2048-wide
ACTIVATEs on ScalarE (the bound), element-wise prep splits DVE/GpSimd.
"""

import sys

for _p in ("/opt/trn_rl_repo",):
    if _p not in sys.path:
        sys.path.insert(0, _p)

import numpy as np
from contextlib import ExitStack

import concourse.bass as bass
import concourse.tile as tile
from concourse import mybir
from concourse.vector_clock import ScopedClock as _ScopedClock


def _patched_drain_and_barrier(self, tick_clock, wait_clock):
    """Tile's closing drain carries one sem-wait per DMA lane used, but this
    walrus build only accepts a single sync wait on a Drain (CTRL-NO)
    lowering ("Too many sync wait commands").  Split the waits across a
    chain of drains (sequential on SP, so semantics are unchanged)."""
    nc = self.nc
    drain_inst = nc.sync.drain()
    wait_clock.add_sem_waits(
        drain_inst.ins, _ScopedClock({None: tick_clock.global_clock})
    )
    si = drain_inst.ins.sync_info
    if si is not None:
        waits = list(si.on_wait or [])
        if len(waits) > 1:
            import bass_rust as _br

            si.on_wait = waits[:1]
            for w in waits[1:]:
                d2 = nc.sync.drain()
                d2.ins.sync_info = _br.SyncInfo(on_wait=[w], on_update=[])
    nc.all_engine_barrier()
    assert self.sems is not None
    popped = nc._tile_sem_poison_stack.pop()
    assert popped is self._sem_poison
    nc.clear_and_free_semaphores(list(self.sems.allocated().values()))
    nc.all_engine_barrier()


tile.TileContext._drain_and_barrier = _patched_drain_and_barrier

_orig_lower_ordered = tile.TileContext._lower_ordered_insts


def _split_multiwaits_and_lower(self, ordered):
    """Same walrus limitation as above, for scheduled compute/DMA
    instructions: hoist all but one sync wait onto single-wait NoOps that
    precede the instruction on its own engine."""
    nc = self.nc
    for insts in ordered.values():
        if not any(
            inst.sync_info is not None and len(inst.sync_info.on_wait or []) > 1
            for inst in insts
        ):
            continue
        out = []
        for inst in insts:
            si = inst.sync_info
            waits = list(si.on_wait) if si is not None and si.on_wait else []
            if len(waits) > 1 and getattr(inst, "engine", None) is not None:
                for w in waits[:-1]:
                    out.append(
                        mybir.InstNoOp(
                            name=nc.get_next_instruction_name(),
                            sync_info=mybir.SyncInfo(on_wait=[w], on_update=[]),
                            bass_nofuse=True,
                            engine=inst.engine,
                        )
                    )
                si.on_wait = waits[-1:]
            out.append(inst)
        insts[:] = out
    return _orig_lower_ordered(self, ordered)


tile.TileContext._lower_ordered_insts = _split_multiwaits_and_lower

N_CORES = 8
N_FULL = 4096
D_FULL = 256

f32 = mybir.dt.float32
f16 = mybir.dt.float16
bf16 = mybir.dt.bfloat16
fp8 = mybir.dt.float8e4
ALU = mybir.AluOpType
AF = mybir.ActivationFunctionType
DR = mybir.MatmulPerfMode.DoubleRow

# descending 16-tile chunks of the global column range [8, 64)
CHUNKS = [(48, 16), (32, 16), (16, 16), (8, 8)]


def strip_cols(si):
    """Suffix strip si covers global column tiles [8*(si+1), 64)."""
    return 8 * (si + 1)


def build_bass(N=N_FULL, D=D_FULL, n_cores=N_CORES):
    n2 = 2 * N
    TF = n2 // 128            # 64 global tiles
    KH = D // 128             # 2 contraction halves
    NR = 16                   # zrows tiles: 8 super-diag + 8 strip rows
    assert D == 256 and TF == 64

    nc = bass.Bass()
    z_i = nc.declare_dram_parameter("z_i", [N, D], f32, isOutput=False)
    z_j = nc.declare_dram_parameter("z_j", [N, D], f32, isOutput=False)
    zr = nc.declare_dram_parameter("zrows", [NR * 128, D], f32, isOutput=False)
    eacc_out = nc.declare_dram_parameter("eacc", [128, n2], bf16, isOutput=True)
    eint_out = nc.declare_dram_parameter("eint", [128, 1024], bf16, isOutput=True)
    sp_out = nc.declare_dram_parameter("spart", [128, 36], f32, isOutput=True)
    pos_out = nc.declare_dram_parameter("posr", [128, 8], f32, isOutput=True)

    with ExitStack() as ctx:
        tc = ctx.enter_context(tile.TileContext(nc))
        big = ctx.enter_context(tc.tile_pool(name="big", bufs=1))
        zfs = ctx.enter_context(tc.tile_pool(name="zfs", bufs=2))
        zns = ctx.enter_context(tc.tile_pool(name="zns", bufs=2))
        zts = ctx.enter_context(tc.tile_pool(name="zts", bufs=2))
        sqs = ctx.enter_context(tc.tile_pool(name="sqs", bufs=2))
        escr = ctx.enter_context(tc.tile_pool(name="escr", bufs=3))
        pmm = ctx.enter_context(tc.tile_pool(name="pmm", bufs=2, space="PSUM"))

        # transposed reps, (tile, half)-interleaved: znT[d, t, h, p] =
        # zn[t*128+p, h*128+d] -- exactly what one whole-chunk XBAR emits
        znT = big.tile([128, TF, KH, 128], bf16)
        ssq = big.tile([128, TF], f16)
        lnss = big.tile([128, TF], f32)
        invn = big.tile([128, TF], f32)

        znr = big.tile([128, NR, D], bf16)        # zrows normalized (kept: pos)
        zrT = big.tile([128, NR, KH, 128], bf16)
        ssqr = big.tile([128, NR], f16)
        lnssr = big.tile([128, NR], f32)
        invnr = big.tile([128, NR], f32)

        E_acc = big.tile([128, n2], bf16)         # strip colsum partials
        E_int = big.tile([128, 1024], bf16)       # super-diag colsum partials
        Spart = big.tile([128, 36], f32)          # rowsum accum slots
        posr = big.tile([128, 8], f32)

        def prep(nt, src_ap, dst_zn, dst_T, d_ssq, d_lnss, d_invn, qoff,
                 toff, gp_frac):
            """load -> sumsq -> invn -> normalize -> XBAR for nt tiles.
            dst_T gets tiles [toff, toff+nt) in (t, h)-interleaved layout."""
            zf = zfs.tile([128, 16, D], bf16, tag="zf")
            nc.gpsimd.dma_start(out=zf[:, :nt, :], in_=src_ap)
            sq = sqs.tile([128, 16, D], bf16, tag="sq")
            q = slice(qoff, qoff + nt)
            nc.vector.tensor_mul(out=sq[:, :nt, :], in0=zf[:, :nt, :],
                                 in1=zf[:, :nt, :])
            with nc.allow_low_precision("fp16 sumsq: |z|^2~256, rel 5e-4"):
                nc.vector.reduce_sum(out=d_ssq[:, q], in_=sq[:, :nt, :],
                                     axis=mybir.AxisListType.X)
            nc.scalar.activation(out=d_lnss[:, q], in_=d_ssq[:, q], func=AF.Ln)
            nc.scalar.activation(out=d_invn[:, q], in_=d_lnss[:, q],
                                 func=AF.Exp, scale=-0.5)
            zn = dst_zn if dst_zn is not None else zns.tile(
                [128, 16, D], bf16, tag="zn")
            n_gp = int(nt * gp_frac)
            for k in range(nt):
                eng = nc.gpsimd if k < n_gp else nc.vector
                eng.tensor_scalar_mul(
                    out=zn[:, k, :], in0=zf[:, k, :],
                    scalar1=d_invn[:, qoff + k : qoff + k + 1],
                )
            nc.sync.dma_start(
                out=dst_T[:, toff : toff + nt, :, :].rearrange(
                    "p t h d -> p (t h) d"),
                in_=zn[:, 0:nt, :].rearrange("p t d -> p (t d)"),
                transpose=True,
            )

        def sim_exp_group(lhs_T, lhs_slot, t0, w, rhs_T, rhs_t0,
                          sp_slot, e_dst_acc, acc_t0, first_touch):
            """One sim block row: lhsT slot lhs_slot (128 rows), columns
            tiles [t0, t0+w) of rhs_T (tile coords rel. rhs_t0), exp with
            rowsum accum into Spart[:, sp_slot]; colsum pieces into
            e_dst_acc starting at tile acc_t0 (None = skip colsum; int =
            first colsum tile, e-columns before it skipped).  bf16 matmul:
            two K-half passes accumulate in PSUM."""
            cols = w * 128
            ps = pmm.tile([128, 2048], f32, tag="ps")
            for j0 in range(0, w, 4):
                wj = min(4, w - j0)
                ta = t0 - rhs_t0 + j0
                for h in range(KH):
                    nc.tensor.matmul(
                        out=ps[:, j0 * 128 : (j0 + wj) * 128],
                        lhsT=lhs_T[:, lhs_slot, h, :],
                        rhs=rhs_T[:, ta : ta + wj, h, :],
                        start=(h == 0), stop=(h == KH - 1),
                    )
            e = escr.tile([128, 2048], bf16, tag="e")
            nc.scalar.activation(
                out=e[:, :cols], in_=ps[:, :cols], func=AF.Exp, scale=2.0,
                accum_out=Spart[:, sp_slot : sp_slot + 1],
            )
            if acc_t0 is None:
                return
            a = acc_t0          # first colsum tile (>= t0)
            b = t0 + w
            if a >= b:
                return
            eng = nc.vector if (sp_slot % 2 == 0) else nc.gpsimd
            d = e_dst_acc[:, a * 128 : b * 128]
            s = e[:, (a - t0) * 128 : (b - t0) * 128]
            if first_touch:
                eng.tensor_copy(out=d, in_=s)
            else:
                eng.tensor_tensor(out=d, in0=d, in1=s, op=ALU.add)

        # ---- zrows first: slots 0-7 = super-diag rows, 8-15 = strip rows
        prep(16, zr[:, :].rearrange("(t p) d -> p t d", p=128),
             znr, zrT, ssqr, lnssr, invnr, 0, 0, 0.5)

        # ---- super-diagonal block: internal upper triangle over slots 0-7
        for a in range(8):
            w = 8 - a
            sim_exp_group(zrT, a, a, w, zrT, 0, 28 + a,
                          E_int, a + 1, first_touch=(a == 0))

        # ---- stream global chunks high->low; emit ready strip groups ----
        zi_r = z_i[:, :].rearrange("(t p) d -> p t d", p=128)
        zj_r = z_j[:, :].rearrange("(t p) d -> p t d", p=128)
        for (c0, nt) in CHUNKS:
            src = (zj_r[:, c0 - 32 : c0 - 32 + nt, :] if c0 >= 32
                   else zi_r[:, c0 : c0 + nt, :])
            prep(nt, src, None, znT, ssq, lnss, invn, c0, c0, 0.3)
            for si in range(7):
                cs = strip_cols(si)          # 8*(si+1)
                t0 = max(cs, c0)
                t1 = c0 + nt
                if t0 >= t1:
                    continue
                g = t0 // 16                 # group ordinal by 16-grid
                sp_slot = si * 4 + g
                sim_exp_group(zrT, 8 + si, t0, t1 - t0, znT, 0, sp_slot,
                              E_acc, t0, first_touch=(si == 0))

        # ---- positive pairs: strip slots si=0..3 vs si+4 ----
        pmul = big.tile([128, 4, D], bf16)
        nc.vector.tensor_mul(out=pmul[:, :, :], in0=znr[:, 8:12, :],
                             in1=znr[:, 12:16, :])
        nc.vector.reduce_sum(out=posr[:, 0:4], in_=pmul[:, :, :],
                             axis=mybir.AxisListType.X)

        nc.sync.dma_start(out=eacc_out[:, :], in_=E_acc)
        nc.sync.dma_start(out=eint_out[:, :], in_=E_int)
        nc.sync.dma_start(out=sp_out[:, :], in_=Spart)
        nc.sync.dma_start(out=pos_out[:, :], in_=posr)

    return nc


_NC_CACHE = {}


def _get_nc(N=N_FULL, D=D_FULL):
    key = (N, D)
    if key not in _NC_CACHE:
        _NC_CACHE[key] = build_bass(N, D)
    return _NC_CACHE[key]


def make_in_maps(z_i, z_j, n_cores=N_CORES):
    z_i = np.ascontiguousarray(z_i, dtype=np.float32)
    z_j = np.ascontiguousarray(z_j, dtype=np.float32)
    reps = np.concatenate([z_i, z_j], axis=0)
    maps = []
    for c in range(n_cores):
        rows = [reps[(8 * c + a) * 128 : (8 * c + a + 1) * 128] for a in range(8)]
        rows += [reps[(8 * si + c) * 128 : (8 * si + c + 1) * 128]
                 for si in range(8)]
        maps.append({
            "z_i": z_i,
            "z_j": z_j,
            "zrows": np.ascontiguousarray(np.concatenate(rows, axis=0)),
        })
    return maps


def assemble(results, N=N_FULL):
    """Host-side gather + reduction + final ln/mean ("all-reduce")."""
    n2 = 2 * N
    S = np.zeros(n2, dtype=np.float64)
    pos = np.zeros(N, dtype=np.float64)
    for c in range(N_CORES):
        r0 = results[c]
        sp = np.asarray(r0["spart"], dtype=np.float64)     # [128, 36]
        ea = np.asarray(r0["eacc"], dtype=np.float32)
        ei = np.asarray(r0["eint"], dtype=np.float32)
        # super-diag rowsums: slot 28+a -> rows (8c+a)*128+p
        for a in range(8):
            S[(8 * c + a) * 128 : (8 * c + a + 1) * 128] += sp[:, 28 + a]
        # strip rowsums: slots si*4+g -> rows (8si+c)*128+p; only the slots
        # the kernel actually wrote (mirror its chunk loop)
        for si in range(7):
            r = 8 * si + c
            acc = np.zeros(128, dtype=np.float64)
            for (c0, nt) in CHUNKS:
                t0 = max(strip_cols(si), c0)
                if t0 < c0 + nt:
                    acc += sp[:, si * 4 + t0 // 16]
            S[r * 128 : (r + 1) * 128] += acc
        # super-diag colsums: E_int col b*128+q -> row (8c+b)*128+q
        # (local tile 0 has no internal colsum contributions -> unwritten)
        eint_sum = ei[:, 128:].astype(np.float64).sum(axis=0)  # [896]
        S[8 * c * 128 + 128 : 8 * c * 128 + 1024] += eint_sum
        # strip colsums: global columns >= 1024
        S[1024:] += ea[:, 1024:].astype(np.float64).sum(axis=0)
        # pos: strip slot si covers tile 8si+c (si=0..3)
        pr = np.asarray(r0["posr"], dtype=np.float64)
        for si in range(4):
            t = 8 * si + c
            pos[t * 128 : (t + 1) * 128] = pr[:, si]
    S -= np.exp(2.0)  # self-similarity exp(2*|zn_a|^2)
    lse = np.log(S)
    loss = np.mean(lse - 2.0 * np.concatenate([pos, pos]))
    return np.float32(loss)


def _run(z_i, z_j, trace=False, tmpdir=None, **spmd_kwargs):
    from concourse.bass_utils import run_bass_kernel_spmd

    N, D = z_i.shape
    nc = _get_nc(N, D)
    in_maps = make_in_maps(z_i, z_j)
    out = run_bass_kernel_spmd(
        nc, in_maps, list(range(N_CORES)), trace=trace, tmpdir=tmpdir,
        **spmd_kwargs
    )
    return assemble(out.results, N), out


def kernel(z_i, z_j):
    loss, _ = _run(np.asarray(z_i), np.asarray(z_j))
    return loss


if __name__ == "__main__":
    rng = np.random.default_rng(0)
    z_i = rng.standard_normal((N_FULL, D_FULL), dtype=np.float32)
    z_j = rng.standard_normal((N_FULL, D_FULL), dtype=np.float32)
    print(kernel(z_i, z_j))


# revision 24
# speedup vs baseline: 2.0826x; 1.5914x over previous
"""NT-Xent loss kernel for Trainium2 (8 NeuronCores, one SPMD program).

Reference (N=4096, D=256, T=0.5):
    zn = l2norm(rows of [z_i; z_j]); sim = zn @ zn.T
    lse_a = ln sum_{b!=a} exp(2 sim_ab);  pos_a = sim[a, a+-N]
    loss = mean(lse_a - 2 pos_a)

Triangle sharding, core-uniform: on the 64x64 grid of 128x128 sim tiles,
each unordered tile pair is computed once:
  * super-diagonal 8x8-tile block I: internal upper triangle, computed by
    core I against its own row tiles ("zrows" slots 0-7, rhs = itself) --
    no per-core addresses.
  * suffix strips: row tile 8*si + c (core c, "zrows" slot 8+si) x columns
    [8*(si+1), 64): the column START is uniform; core identity lives only
    in the input data.  260 exp-tiles per core, perfectly balanced.
Row sums come from ACTIVATE-exp accumulators; column sums (for the mirrored
lower triangle) accumulate in bf16 E_acc via DVE/GpSimd adds and are
finished on the host together with the final ln/mean ("all-reduce").

Engine plan: transposes go through the DMA XBAR (bf16, 2-byte path), main
matmuls are fp8e4 DoubleRow (K=256 in one pass), exp runs in 2048-wide
ACTIVATEs on ScalarE (the bound), element-wise prep splits DVE/GpSimd.
"""

import sys

for _p in ("/opt/trn_rl_repo",):
    if _p not in sys.path:
        sys.path.insert(0, _p)

import numpy as np
from contextlib import ExitStack

import concourse.bass as bass
import concourse.tile as tile
from concourse import mybir
from concourse.vector_clock import ScopedClock as _ScopedClock


def _patched_drain_and_barrier(self, tick_clock, wait_clock):
    """Tile's closing drain carries one sem-wait per DMA lane used, but this
    walrus build only accepts a single sync wait on a Drain (CTRL-NO)
    lowering ("Too many sync wait commands").  Split the waits across a
    chain of drains (sequential on SP, so semantics are unchanged)."""
    nc = self.nc
    drain_inst = nc.sync.drain()
    wait_clock.add_sem_waits(
        drain_inst.ins, _ScopedClock({None: tick_clock.global_clock})
    )
    si = drain_inst.ins.sync_info
    if si is not None:
        waits = list(si.on_wait or [])
        if len(waits) > 1:
            import bass_rust as _br

            si.on_wait = waits[:1]
            for w in waits[1:]:
                d2 = nc.sync.drain()
                d2.ins.sync_info = _br.SyncInfo(on_wait=[w], on_update=[])
    nc.all_engine_barrier()
    assert self.sems is not None
    popped = nc._tile_sem_poison_stack.pop()
    assert popped is self._sem_poison
    nc.clear_and_free_semaphores(list(self.sems.allocated().values()))
    nc.all_engine_barrier()


tile.TileContext._drain_and_barrier = _patched_drain_and_barrier

_orig_lower_ordered = tile.TileContext._lower_ordered_insts


def _split_multiwaits_and_lower(self, ordered):
    """Same walrus limitation as above, for scheduled compute/DMA
    instructions: hoist all but one sync wait onto single-wait NoOps that
    precede the instruction on its own engine."""
    nc = self.nc
    for insts in ordered.values():
        if not any(
            inst.sync_info is not None and len(inst.sync_info.on_wait or []) > 1
            for inst in insts
        ):
            continue
        out = []
        for inst in insts:
            si = inst.sync_info
            waits = list(si.on_wait) if si is not None and si.on_wait else []
            if len(waits) > 1 and getattr(inst, "engine", None) is not None:
                for w in waits[:-1]:
                    out.append(
                        mybir.InstNoOp(
                            name=nc.get_next_instruction_name(),
                            sync_info=mybir.SyncInfo(on_wait=[w], on_update=[]),
                            bass_nofuse=True,
                            engine=inst.engine,
                        )
                    )
                si.on_wait = waits[-1:]
            out.append(inst)
        insts[:] = out
    return _orig_lower_ordered(self, ordered)


tile.TileContext._lower_ordered_insts = _split_multiwaits_and_lower

N_CORES = 8
N_FULL = 4096
D_FULL = 256

f32 = mybir.dt.float32
f16 = mybir.dt.float16
bf16 = mybir.dt.bfloat16
fp8 = mybir.dt.float8e4
ALU = mybir.AluOpType
AF = mybir.ActivationFunctionType
DR = mybir.MatmulPerfMode.DoubleRow

# descending 16-tile chunks of the global column range [8, 64)
CHUNKS = [(48, 16), (32, 16), (16, 16), (8, 8)]


def strip_cols(si):
    """Suffix strip si covers global column tiles [8*(si+1), 64)."""
    return 8 * (si + 1)


def build_bass(N=N_FULL, D=D_FULL, n_cores=N_CORES):
    n2 = 2 * N
    TF = n2 // 128            # 64 global tiles
    KH = D // 128             # 2 contraction halves
    NR = 16                   # zrows tiles: 8 super-diag + 8 strip rows
    assert D == 256 and TF == 64

    nc = bass.Bass()
    z_i = nc.declare_dram_parameter("z_i", [N, D], f32, isOutput=False)
    z_j = nc.declare_dram_parameter("z_j", [N, D], f32, isOutput=False)
    zr = nc.declare_dram_parameter("zrows", [NR * 128, D], f32, isOutput=False)
    eacc_out = nc.declare_dram_parameter("eacc", [128, n2], bf16, isOutput=True)
    eint_out = nc.declare_dram_parameter("eint", [128, 1024], bf16, isOutput=True)
    sp_out = nc.declare_dram_parameter("spart", [128, 36], f32, isOutput=True)
    pos_out = nc.declare_dram_parameter("posr", [128, 8], f32, isOutput=True)

    with ExitStack() as ctx:
        tc = ctx.enter_context(tile.TileContext(nc))
        big = ctx.enter_context(tc.tile_pool(name="big", bufs=1))
        zfs = ctx.enter_context(tc.tile_pool(name="zfs", bufs=2))
        zns = ctx.enter_context(tc.tile_pool(name="zns", bufs=2))
        zts = ctx.enter_context(tc.tile_pool(name="zts", bufs=2))
        sqs = ctx.enter_context(tc.tile_pool(name="sqs", bufs=2))
        escr = ctx.enter_context(tc.tile_pool(name="escr", bufs=3))
        pmm = ctx.enter_context(tc.tile_pool(name="pmm", bufs=2, space="PSUM"))

        # transposed reps, (tile, half)-interleaved: znT[d, t, h, p] =
        # zn[t*128+p, h*128+d] -- exactly what one whole-chunk XBAR emits
        znT = big.tile([128, TF, KH, 128], bf16)
        ssq = big.tile([128, TF], f16)
        lnss = big.tile([128, TF], f32)
        invn = big.tile([128, TF], f32)

        znr = big.tile([128, NR, D], bf16)        # zrows normalized (kept: pos)
        zrT = big.tile([128, NR, KH, 128], bf16)
        ssqr = big.tile([128, NR], f16)
        lnssr = big.tile([128, NR], f32)
        invnr = big.tile([128, NR], f32)

        E_acc = big.tile([128, n2], bf16)         # strip colsum partials
        E_int = big.tile([128, 1024], bf16)       # super-diag colsum partials
        Spart = big.tile([128, 36], f32)          # rowsum accum slots
        posr = big.tile([128, 8], f32)

        def prep(nt, src_ap, dst_zn, dst_T, d_ssq, d_lnss, d_invn, qoff,
                 toff, gp_frac):
            """load -> sumsq -> invn -> normalize -> XBAR for nt tiles.
            dst_T gets tiles [toff, toff+nt) in (t, h)-interleaved layout."""
            zf = zfs.tile([128, 16, D], bf16, tag="zf")
            nc.gpsimd.dma_start(out=zf[:, :nt, :], in_=src_ap)
            sq = sqs.tile([128, 16, D], bf16, tag="sq")
            q = slice(qoff, qoff + nt)
            nc.vector.tensor_mul(out=sq[:, :nt, :], in0=zf[:, :nt, :],
                                 in1=zf[:, :nt, :])
            with nc.allow_low_precision("fp16 sumsq: |z|^2~256, rel 5e-4"):
                nc.vector.reduce_sum(out=d_ssq[:, q], in_=sq[:, :nt, :],
                                     axis=mybir.AxisListType.X)
            nc.scalar.activation(out=d_lnss[:, q], in_=d_ssq[:, q], func=AF.Ln)
            nc.scalar.activation(out=d_invn[:, q], in_=d_lnss[:, q],
                                 func=AF.Exp, scale=-0.5)
            zn = dst_zn if dst_zn is not None else zns.tile(
                [128, 16, D], bf16, tag="zn")
            for k in range(nt):
                # Q7 tensor_scalar measured ~4us/tile -- keep all on DVE
                nc.vector.tensor_scalar_mul(
                    out=zn[:, k, :], in0=zf[:, k, :],
                    scalar1=d_invn[:, qoff + k : qoff + k + 1],
                )
            nc.sync.dma_start(
                out=dst_T[:, toff : toff + nt, :, :].rearrange(
                    "p t h d -> p (t h) d"),
                in_=zn[:, 0:nt, :].rearrange("p t d -> p (t d)"),
                transpose=True,
            )

        def sim_exp_group(lhs_T, lhs_slot, t0, w, rhs_T, rhs_t0,
                          sp_slot, e_dst_acc, acc_t0, first_touch):
            """One sim block row: lhsT slot lhs_slot (128 rows), columns
            tiles [t0, t0+w) of rhs_T (tile coords rel. rhs_t0), exp with
            rowsum accum into Spart[:, sp_slot]; colsum pieces into
            e_dst_acc starting at tile acc_t0 (None = skip colsum; int =
            first colsum tile, e-columns before it skipped).  bf16 matmul:
            two K-half passes accumulate in PSUM."""
            cols = w * 128
            ps = pmm.tile([128, 2048], f32, tag="ps")
            for j0 in range(0, w, 4):
                wj = min(4, w - j0)
                ta = t0 - rhs_t0 + j0
                for h in range(KH):
                    nc.tensor.matmul(
                        out=ps[:, j0 * 128 : (j0 + wj) * 128],
                        lhsT=lhs_T[:, lhs_slot, h, :],
                        rhs=rhs_T[:, ta : ta + wj, h, :],
                        start=(h == 0), stop=(h == KH - 1),
                    )
            e = escr.tile([128, 2048], bf16, tag="e")
            nc.scalar.activation(
                out=e[:, :cols], in_=ps[:, :cols], func=AF.Exp, scale=2.0,
                accum_out=Spart[:, sp_slot : sp_slot + 1],
            )
            if acc_t0 is None:
                return
            a = acc_t0          # first colsum tile (>= t0)
            b = t0 + w
            if a >= b:
                return
            d = e_dst_acc[:, a * 128 : b * 128]
            s = e[:, (a - t0) * 128 : (b - t0) * 128]
            if first_touch:
                nc.vector.tensor_copy(out=d, in_=s)  # Q7 COPY is ~7us: DVE
            else:
                # split adds ~2:1 DVE:Q7 by alternating on slot parity
                eng = nc.gpsimd if sp_slot % 3 == 2 else nc.vector
                eng.tensor_tensor(out=d, in0=d, in1=s, op=ALU.add)

        # ---- zrows first: slots 0-7 = super-diag rows, 8-15 = strip rows
        prep(16, zr[:, :].rearrange("(t p) d -> p t d", p=128),
             znr, zrT, ssqr, lnssr, invnr, 0, 0, 0.5)

        zi_r = z_i[:, :].rearrange("(t p) d -> p t d", p=128)
        zj_r = z_j[:, :].rearrange("(t p) d -> p t d", p=128)

        def emit_prep(step):
            c0, nt = CHUNKS[step]
            src = (zj_r[:, c0 - 32 : c0 - 32 + nt, :] if c0 >= 32
                   else zi_r[:, c0 : c0 + nt, :])
            prep(nt, src, None, znT, ssq, lnss, invn, c0, c0, 0.3)

        def emit_groups(step):
            c0, nt = CHUNKS[step]
            for si in range(7):
                t0 = max(strip_cols(si), c0)
                t1 = c0 + nt
                if t0 >= t1:
                    continue
                g = t0 // 16                 # group ordinal by 16-grid
                sp_slot = si * 4 + g
                sim_exp_group(zrT, 8 + si, t0, t1 - t0, znT, 0, sp_slot,
                              E_acc, t0, first_touch=(si == 0))

        # software-pipelined emission: each chunk's prep is queued ahead of
        # the previous chunk's consumers so per-engine in-order queues never
        # block a prep behind main-loop work.
        emit_prep(0)
        # super-diagonal block: internal upper triangle over zrows slots 0-7
        for a in range(8):
            sim_exp_group(zrT, a, a, 8 - a, zrT, 0, 28 + a,
                          E_int, a + 1, first_touch=(a == 0))
        emit_prep(1)
        emit_groups(0)
        emit_prep(2)
        emit_groups(1)
        emit_prep(3)
        emit_groups(2)
        emit_groups(3)

        # ---- positive pairs: strip slots si=0..3 vs si+4 ----
        pmul = big.tile([128, 4, D], bf16)
        nc.vector.tensor_mul(out=pmul[:, :, :], in0=znr[:, 8:12, :],
                             in1=znr[:, 12:16, :])
        nc.vector.reduce_sum(out=posr[:, 0:4], in_=pmul[:, :, :],
                             axis=mybir.AxisListType.X)

        nc.sync.dma_start(out=eacc_out[:, :], in_=E_acc)
        nc.sync.dma_start(out=eint_out[:, :], in_=E_int)
        nc.sync.dma_start(out=sp_out[:, :], in_=Spart)
        nc.sync.dma_start(out=pos_out[:, :], in_=posr)

    return nc


_NC_CACHE = {}


def _get_nc(N=N_FULL, D=D_FULL):
    key = (N, D)
    if key not in _NC_CACHE:
        _NC_CACHE[key] = build_bass(N, D)
    return _NC_CACHE[key]


def make_in_maps(z_i, z_j, n_cores=N_CORES):
    z_i = np.ascontiguousarray(z_i, dtype=np.float32)
    z_j = np.ascontiguousarray(z_j, dtype=np.float32)
    reps = np.concatenate([z_i, z_j], axis=0)
    maps = []
    for c in range(n_cores):
        rows = [reps[(8 * c + a) * 128 : (8 * c + a + 1) * 128] for a in range(8)]
        rows += [reps[(8 * si + c) * 128 : (8 * si + c + 1) * 128]
                 for si in range(8)]
        maps.append({
            "z_i": z_i,
            "z_j": z_j,
            "zrows": np.ascontiguousarray(np.concatenate(rows, axis=0)),
        })
    return maps


def assemble(results, N=N_FULL):
    """Host-side gather + reduction + final ln/mean ("all-reduce")."""
    n2 = 2 * N
    S = np.zeros(n2, dtype=np.float64)
    pos = np.zeros(N, dtype=np.float64)
    for c in range(N_CORES):
        r0 = results[c]
        sp = np.asarray(r0["spart"], dtype=np.float64)     # [128, 36]
        ea = np.asarray(r0["eacc"], dtype=np.float32)
        ei = np.asarray(r0["eint"], dtype=np.float32)
        # super-diag rowsums: slot 28+a -> rows (8c+a)*128+p
        for a in range(8):
            S[(8 * c + a) * 128 : (8 * c + a + 1) * 128] += sp[:, 28 + a]
        # strip rowsums: slots si*4+g -> rows (8si+c)*128+p; only the slots
        # the kernel actually wrote (mirror its chunk loop)
        for si in range(7):
            r = 8 * si + c
            acc = np.zeros(128, dtype=np.float64)
            for (c0, nt) in CHUNKS:
                t0 = max(strip_cols(si), c0)
                if t0 < c0 + nt:
                    acc += sp[:, si * 4 + t0 // 16]
            S[r * 128 : (r + 1) * 128] += acc
        # super-diag colsums: E_int col b*128+q -> row (8c+b)*128+q
        # (local tile 0 has no internal colsum contributions -> unwritten)
        eint_sum = ei[:, 128:].astype(np.float64).sum(axis=0)  # [896]
        S[8 * c * 128 + 128 : 8 * c * 128 + 1024] += eint_sum
        # strip colsums: global columns >= 1024
        S[1024:] += ea[:, 1024:].astype(np.float64).sum(axis=0)
        # pos: strip slot si covers tile 8si+c (si=0..3)
        pr = np.asarray(r0["posr"], dtype=np.float64)
        for si in range(4):
            t = 8 * si + c
            pos[t * 128 : (t + 1) * 128] = pr[:, si]
    S -= np.exp(2.0)  # self-similarity exp(2*|zn_a|^2)
    lse = np.log(S)
    loss = np.mean(lse - 2.0 * np.concatenate([pos, pos]))
    return np.float32(loss)


def _run(z_i, z_j, trace=False, tmpdir=None, **spmd_kwargs):
    from concourse.bass_utils import run_bass_kernel_spmd

    N, D = z_i.shape
    nc = _get_nc(N, D)
    in_maps = make_in_maps(z_i, z_j)
    out = run_bass_kernel_spmd(
        nc, in_maps, list(range(N_CORES)), trace=trace, tmpdir=tmpdir,
        **spmd_kwargs
    )
    return assemble(out.results, N), out


def kernel(z_i, z_j):
    loss, _ = _run(np.asarray(z_i), np.asarray(z_j))
    return loss


if __name__ == "__main__":
    rng = np.random.default_rng(0)
    z_i = rng.standard_normal((N_FULL, D_FULL), dtype=np.float32)
    z_j = rng.standard_normal((N_FULL, D_FULL), dtype=np.float32)
    print(kernel(z_i, z_j))


# revision 28
# speedup vs baseline: 2.1883x; 1.0508x over previous
"""NT-Xent loss kernel for Trainium2 (8 NeuronCores, one SPMD program).

Reference (N=4096, D=256, T=0.5):
    zn = l2norm(rows of [z_i; z_j]); sim = zn @ zn.T
    lse_a = ln sum_{b!=a} exp(2 sim_ab);  pos_a = sim[a, a+-N]
    loss = mean(lse_a - 2 pos_a)

Triangle sharding, core-uniform: on the 64x64 grid of 128x128 sim tiles,
each unordered tile pair is computed once:
  * super-diagonal 8x8-tile block I: internal upper triangle, computed by
    core I against its own row tiles ("zrows" slots 0-7, rhs = itself).
  * suffix strips: row tile 8*si + c (core c, "zrows" slot 8+si) x columns
    [8*(si+1), 64): the column START is uniform across cores; core identity
    lives only in the input data.  260 exp-tiles per core, balanced.
Row sums come from ACTIVATE-exp accumulators; column sums (mirrored lower
triangle) accumulate in bf16 E_acc/E_int via DVE/GpSimd adds; the host
finishes the reduction and the final ln/mean ("all-reduce").

Engine plan:
  * transposes on the PE (is_transpose matmul, bf16 -> bf16 PSUM), eight
    128x128 tile-halves per PSUM bank; GpSimd copies PSUM -> SBUF as fp8.
    (The DMA XBAR path measures ~82 GB/s and serializes against loads --
    too slow for the 4.5 MB of transposed reps.)
  * main matmuls in fp8e4 DoubleRow (K=256 folded into one pass).
  * exp in <=1536-wide ACTIVATEs on ScalarE (PSUM: 2x3 sim banks + 2
    transpose banks), rowsum via accum_out.
  * element-wise prep (sumsq, invn scale, colsum adds) on DVE; GpSimd adds
    take a minority share.  Emission is software-pipelined per engine so
    in-order queues never park a prep behind main-loop consumers.
"""

import sys

for _p in ("/opt/trn_rl_repo",):
    if _p not in sys.path:
        sys.path.insert(0, _p)

import numpy as np
from contextlib import ExitStack

import concourse.bass as bass
import concourse.tile as tile
from concourse import mybir
from concourse.masks import make_identity
from concourse.vector_clock import ScopedClock as _ScopedClock


def _patched_drain_and_barrier(self, tick_clock, wait_clock):
    """Tile's closing drain carries one sem-wait per DMA lane used, but this
    walrus build only accepts a single sync wait on a Drain (CTRL-NO)
    lowering ("Too many sync wait commands").  Split the waits across a
    chain of drains (sequential on SP, so semantics are unchanged)."""
    nc = self.nc
    drain_inst = nc.sync.drain()
    wait_clock.add_sem_waits(
        drain_inst.ins, _ScopedClock({None: tick_clock.global_clock})
    )
    si = drain_inst.ins.sync_info
    if si is not None:
        waits = list(si.on_wait or [])
        if len(waits) > 1:
            import bass_rust as _br

            si.on_wait = waits[:1]
            for w in waits[1:]:
                d2 = nc.sync.drain()
                d2.ins.sync_info = _br.SyncInfo(on_wait=[w], on_update=[])
    nc.all_engine_barrier()
    assert self.sems is not None
    popped = nc._tile_sem_poison_stack.pop()
    assert popped is self._sem_poison
    nc.clear_and_free_semaphores(list(self.sems.allocated().values()))
    nc.all_engine_barrier()


tile.TileContext._drain_and_barrier = _patched_drain_and_barrier

_orig_lower_ordered = tile.TileContext._lower_ordered_insts


def _split_multiwaits_and_lower(self, ordered):
    """Same walrus limitation as above, for scheduled compute/DMA
    instructions: hoist all but one sync wait onto single-wait NoOps that
    precede the instruction on its own engine."""
    nc = self.nc
    for insts in ordered.values():
        if not any(
            inst.sync_info is not None and len(inst.sync_info.on_wait or []) > 1
            for inst in insts
        ):
            continue
        out = []
        for inst in insts:
            si = inst.sync_info
            waits = list(si.on_wait) if si is not None and si.on_wait else []
            if len(waits) > 1 and getattr(inst, "engine", None) is not None:
                for w in waits[:-1]:
                    out.append(
                        mybir.InstNoOp(
                            name=nc.get_next_instruction_name(),
                            sync_info=mybir.SyncInfo(on_wait=[w], on_update=[]),
                            bass_nofuse=True,
                            engine=inst.engine,
                        )
                    )
                si.on_wait = waits[-1:]
            out.append(inst)
        insts[:] = out
    return _orig_lower_ordered(self, ordered)


tile.TileContext._lower_ordered_insts = _split_multiwaits_and_lower

N_CORES = 8
N_FULL = 4096
D_FULL = 256

f32 = mybir.dt.float32
f16 = mybir.dt.float16
bf16 = mybir.dt.bfloat16
fp8 = mybir.dt.float8e4
ALU = mybir.AluOpType
AF = mybir.ActivationFunctionType
DR = mybir.MatmulPerfMode.DoubleRow

# descending 16-tile chunks of the global column range [8, 64)
CHUNKS = [(48, 16), (32, 16), (16, 16), (8, 8)]
GW = 12  # max ACT group width in tiles (PSUM sim tile = 1536 f32 = 3 banks)


def plan_groups():
    """Emission-ordered sim/exp groups shared by build_bass and assemble.

    Returns a list of dicts: owner ('int', a) or ('strip', si); lhs slot in
    zrows; t0/w column tiles (zrows-local for 'int', global otherwise);
    sp_slot; colsum first-touch flag; acc_t0 (first colsum tile or None).
    """
    groups = []
    slot = 0

    def add(owner, lhs_slot, t0, w, acc_t0, ft):
        nonlocal slot
        groups.append(dict(owner=owner, lhs=lhs_slot, t0=t0, w=w,
                           acc_t0=acc_t0, ft=ft, slot=slot))
        slot += 1

    # internal triangle of the super-diagonal block (zrows slots 0-7)
    for a in range(8):
        add(("int", a), a, a, 8 - a, a + 1, a == 0)
    # suffix strips, chunk-streamed, split to <= GW tiles
    for step, (c0, nt) in enumerate(CHUNKS):
        for si in range(7):
            t0 = max(8 * (si + 1), c0)
            t1 = c0 + nt
            p0 = t0
            while p0 < t1:
                w = min(GW, t1 - p0)
                add(("strip", si), 8 + si, p0, w, p0, si == 0)
                p0 += w
    return groups


GROUPS = plan_groups()
N_SLOTS = len(GROUPS)


def build_bass(N=N_FULL, D=D_FULL, n_cores=N_CORES):
    n2 = 2 * N
    TF = n2 // 128            # 64 global tiles
    KH = D // 128             # 2 contraction halves
    NR = 16                   # zrows tiles: 8 super-diag + 8 strip rows
    assert D == 256 and TF == 64

    nc = bass.Bass()
    z_i = nc.declare_dram_parameter("z_i", [N, D], f32, isOutput=False)
    z_j = nc.declare_dram_parameter("z_j", [N, D], f32, isOutput=False)
    zr = nc.declare_dram_parameter("zrows", [NR * 128, D], f32, isOutput=False)
    eacc_out = nc.declare_dram_parameter("eacc", [128, n2], bf16, isOutput=True)
    eint_out = nc.declare_dram_parameter("eint", [128, 1024], bf16, isOutput=True)
    sp_out = nc.declare_dram_parameter("spart", [128, N_SLOTS], f32, isOutput=True)
    pos_out = nc.declare_dram_parameter("posr", [128, 8], f32, isOutput=True)

    with ExitStack() as ctx:
        tc = ctx.enter_context(tile.TileContext(nc))
        big = ctx.enter_context(tc.tile_pool(name="big", bufs=1))
        zfs = ctx.enter_context(tc.tile_pool(name="zfs", bufs=3))
        zns = ctx.enter_context(tc.tile_pool(name="zns", bufs=2))
        sqs = ctx.enter_context(tc.tile_pool(name="sqs", bufs=2))
        escr = ctx.enter_context(tc.tile_pool(name="escr", bufs=3))
        pmm = ctx.enter_context(tc.tile_pool(name="pmm", bufs=2, space="PSUM"))
        pmt = ctx.enter_context(tc.tile_pool(name="pmt", bufs=2, space="PSUM"))

        ident = big.tile([128, 128], bf16)
        make_identity(nc, ident)

        znT8 = big.tile([128, KH, n2], fp8)       # global transposed fp8
        ssq = big.tile([128, TF], f16)
        lnss = big.tile([128, TF], f32)
        invn = big.tile([128, TF], f32)

        znr = big.tile([128, NR, D], bf16)        # zrows normalized (pos)
        zrT8 = big.tile([128, KH, NR * 128], fp8)
        ssqr = big.tile([128, NR], f16)
        lnssr = big.tile([128, NR], f32)
        invnr = big.tile([128, NR], f32)

        E_acc = big.tile([128, n2], bf16)         # strip colsum partials
        E_int = big.tile([128, 1024], bf16)       # super-diag colsum partials
        Spart = big.tile([128, N_SLOTS], f32)     # rowsum accum slots
        posr = big.tile([128, 8], f32)

        zi_r = z_i[:, :].rearrange("(t p) d -> p t d", p=128)
        zj_r = z_j[:, :].rearrange("(t p) d -> p t d", p=128)

        # ---- all loads issued up front; SWDGE queue + pool WAR deps give
        # natural flow control (zfs bufs=3)
        zf_tiles = []
        zf_r = zfs.tile([128, 16, D], bf16, tag="zf")
        nc.gpsimd.dma_start(out=zf_r[:, :, :],
                            in_=zr[:, :].rearrange("(t p) d -> p t d", p=128))
        for (c0, nt) in CHUNKS:
            src = (zj_r[:, c0 - 32 : c0 - 32 + nt, :] if c0 >= 32
                   else zi_r[:, c0 : c0 + nt, :])
            zf = zfs.tile([128, 16, D], bf16, tag="zf")
            nc.gpsimd.dma_start(out=zf[:, :nt, :], in_=src)
            zf_tiles.append(zf)

        def prep_dve(zf, nt, dst_zn, d_ssq, d_lnss, d_invn, qoff):
            """sumsq -> invn -> normalize (DVE + ScalarE only)."""
            sq = sqs.tile([128, 16, D], bf16, tag="sq")
            q = slice(qoff, qoff + nt)
            nc.vector.tensor_mul(out=sq[:, :nt, :], in0=zf[:, :nt, :],
                                 in1=zf[:, :nt, :])
            with nc.allow_low_precision("fp16 sumsq: |z|^2~256, rel 5e-4"):
                nc.vector.reduce_sum(out=d_ssq[:, q], in_=sq[:, :nt, :],
                                     axis=mybir.AxisListType.X)
            nc.scalar.activation(out=d_lnss[:, q], in_=d_ssq[:, q], func=AF.Ln)
            nc.scalar.activation(out=d_invn[:, q], in_=d_lnss[:, q],
                                 func=AF.Exp, scale=-0.5)
            zn = dst_zn if dst_zn is not None else zns.tile(
                [128, 16, D], bf16, tag="zn")
            for k in range(nt):
                nc.vector.tensor_scalar_mul(
                    out=zn[:, k, :], in0=zf[:, k, :],
                    scalar1=d_invn[:, qoff + k : qoff + k + 1],
                )
            return zn

        copy_flip = [0]

        def prep_pe(zn, nt, dst_T8, toff):
            """PE tile-half transposes -> PSUM bf16 -> fp8 copies.  GpSimd
            cannot read PSUM; alternate the copies DVE/ScalarE."""
            for h in range(KH):
                for g0 in range(0, nt, 8):
                    gn = min(8, nt - g0)
                    pt = pmt.tile([128, 1024], bf16, tag="pt")
                    for k in range(gn):
                        nc.tensor.matmul(
                            out=pt[:, k * 128 : (k + 1) * 128],
                            lhsT=zn[:, g0 + k, h * 128 : (h + 1) * 128],
                            rhs=ident, is_transpose=True,
                        )
                    dst = dst_T8[:, h,
                                 (toff + g0) * 128 : (toff + g0 + gn) * 128]
                    if copy_flip[0] % 3 == 2:
                        nc.scalar.copy(out=dst, in_=pt[:, : gn * 128])
                    else:
                        nc.vector.tensor_copy(out=dst, in_=pt[:, : gn * 128])
                    copy_flip[0] += 1

        def sim_exp_group(g):
            """MMs -> exp(+rowsum accum) -> colsum accumulate for one group."""
            t0, w, slot = g["t0"], g["w"], g["slot"]
            internal = g["owner"][0] == "int"
            lhs = zrT8
            rhs = zrT8 if internal else znT8
            cols = w * 128
            ps = pmm.tile([128, GW * 128], f32, tag="ps")
            for j0 in range(0, w, 4):
                wj = min(4, w - j0) * 128
                c0 = (t0 + j0) * 128
                nc.tensor.matmul(
                    out=ps[:, j0 * 128 : j0 * 128 + wj],
                    lhsT=lhs[:, :, g["lhs"] * 128 : (g["lhs"] + 1) * 128],
                    rhs=rhs[:, :, c0 : c0 + wj],
                    start=True, stop=True, perf_mode=DR,
                )
            dst_buf = E_int if internal else E_acc
            if g["ft"]:
                # first touch of these E columns: the exp output IS the
                # colsum partial -- write it there directly, no copy.
                # (E columns left of acc_t0 get written too; the host
                # ignores them for 'int' and they are never first-touch
                # overlaps for strips since si=0 spans every range.)
                e = dst_buf[:, t0 * 128 : (t0 + w) * 128]
            else:
                e_t = escr.tile([128, GW * 128], bf16, tag="e")
                e = e_t[:, :cols]
            nc.scalar.activation(
                out=e, in_=ps[:, :cols], func=AF.Exp, scale=2.0,
                accum_out=Spart[:, slot : slot + 1],
            )
            if g["ft"]:
                return
            a, b = g["acc_t0"], t0 + w
            if a >= b:
                return
            d = dst_buf[:, a * 128 : b * 128]
            s = e[:, (a - t0) * 128 : (b - t0) * 128]
            eng = nc.gpsimd if slot % 2 == 1 else nc.vector
            eng.tensor_tensor(out=d, in0=d, in1=s, op=ALU.add)

        # group emission bookkeeping
        int_groups = [g for g in GROUPS if g["owner"][0] == "int"]
        step_groups = [[] for _ in CHUNKS]
        for g in GROUPS:
            if g["owner"][0] == "strip":
                for step, (c0, nt) in enumerate(CHUNKS):
                    if c0 <= g["t0"] < c0 + nt:
                        step_groups[step].append(g)

        # ---- software-pipelined emission ----
        znr_t = prep_dve(zf_r, 16, znr, ssqr, lnssr, invnr, 0)
        prep_pe(znr_t, 16, zrT8, 0)
        zn0 = prep_dve(zf_tiles[0], CHUNKS[0][1], None, ssq, lnss, invn,
                       CHUNKS[0][0])
        for g in int_groups:
            sim_exp_group(g)
        prep_pe(zn0, CHUNKS[0][1], znT8, CHUNKS[0][0])
        for step in range(len(CHUNKS)):
            if step + 1 < len(CHUNKS):
                c0, nt = CHUNKS[step + 1]
                zn1 = prep_dve(zf_tiles[step + 1], nt, None, ssq, lnss, invn,
                               c0)
            for g in step_groups[step]:
                sim_exp_group(g)
            if step + 1 < len(CHUNKS):
                prep_pe(zn1, CHUNKS[step + 1][1], znT8, CHUNKS[step + 1][0])

        # ---- positive pairs: zrows strip slots si=0..3 vs si+4 ----
        pmul = big.tile([128, 4, D], bf16)
        nc.vector.tensor_mul(out=pmul[:, :, :], in0=znr[:, 8:12, :],
                             in1=znr[:, 12:16, :])
        nc.vector.reduce_sum(out=posr[:, 0:4], in_=pmul[:, :, :],
                             axis=mybir.AxisListType.X)

        nc.sync.dma_start(out=eacc_out[:, :], in_=E_acc)
        nc.sync.dma_start(out=eint_out[:, :], in_=E_int)
        nc.sync.dma_start(out=sp_out[:, :], in_=Spart)
        nc.sync.dma_start(out=pos_out[:, :], in_=posr)

    return nc


_NC_CACHE = {}


def _get_nc(N=N_FULL, D=D_FULL):
    key = (N, D)
    if key not in _NC_CACHE:
        _NC_CACHE[key] = build_bass(N, D)
    return _NC_CACHE[key]


def make_in_maps(z_i, z_j, n_cores=N_CORES):
    z_i = np.ascontiguousarray(z_i, dtype=np.float32)
    z_j = np.ascontiguousarray(z_j, dtype=np.float32)
    reps = np.concatenate([z_i, z_j], axis=0)
    maps = []
    for c in range(n_cores):
        rows = [reps[(8 * c + a) * 128 : (8 * c + a + 1) * 128] for a in range(8)]
        rows += [reps[(8 * si + c) * 128 : (8 * si + c + 1) * 128]
                 for si in range(8)]
        maps.append({
            "z_i": z_i,
            "z_j": z_j,
            "zrows": np.ascontiguousarray(np.concatenate(rows, axis=0)),
        })
    return maps


def assemble(results, N=N_FULL):
    """Host-side gather + reduction + final ln/mean ("all-reduce")."""
    n2 = 2 * N
    S = np.zeros(n2, dtype=np.float64)
    pos = np.zeros(N, dtype=np.float64)
    for c in range(N_CORES):
        r0 = results[c]
        sp = np.asarray(r0["spart"], dtype=np.float64)
        ea = np.asarray(r0["eacc"], dtype=np.float32)
        ei = np.asarray(r0["eint"], dtype=np.float32)
        for g in GROUPS:
            kind, idx = g["owner"]
            r = 8 * c + idx if kind == "int" else 8 * idx + c
            S[r * 128 : (r + 1) * 128] += sp[:, g["slot"]]
        # super-diag colsums: E_int col b*128+q -> row (8c+b)*128+q
        eint_sum = ei[:, 128:].astype(np.float64).sum(axis=0)
        S[8 * c * 128 + 128 : 8 * c * 128 + 1024] += eint_sum
        # strip colsums: global columns >= 1024
        S[1024:] += ea[:, 1024:].astype(np.float64).sum(axis=0)
        # pos: strip slot si covers tile 8si+c (si=0..3)
        pr = np.asarray(r0["posr"], dtype=np.float64)
        for si in range(4):
            t = 8 * si + c
            pos[t * 128 : (t + 1) * 128] = pr[:, si]
    S -= np.exp(2.0)  # self-similarity exp(2*|zn_a|^2)
    lse = np.log(S)
    loss = np.mean(lse - 2.0 * np.concatenate([pos, pos]))
    return np.float32(loss)


def _run(z_i, z_j, trace=False, tmpdir=None, **spmd_kwargs):
    from concourse.bass_utils import run_bass_kernel_spmd

    N, D = z_i.shape
    nc = _get_nc(N, D)
    in_maps = make_in_maps(z_i, z_j)
    out = run_bass_kernel_spmd(
        nc, in_maps, list(range(N_CORES)), trace=trace, tmpdir=tmpdir,
        **spmd_kwargs
    )
    return assemble(out.results, N), out


def kernel(z_i, z_j):
    loss, _ = _run(np.asarray(z_i), np.asarray(z_j))
    return loss


if __name__ == "__main__":
    rng = np.random.default_rng(0)
    z_i = rng.standard_normal((N_FULL, D_FULL), dtype=np.float32)
    z_j = rng.standard_normal((N_FULL, D_FULL), dtype=np.float32)
    print(kernel(z_i, z_j))


# revision 30
# speedup vs baseline: 2.2082x; 1.0091x over previous
"""NT-Xent loss kernel for Trainium2 (8 NeuronCores, one SPMD program).

Reference (N=4096, D=256, T=0.5):
    zn = l2norm(rows of [z_i; z_j]); sim = zn @ zn.T
    lse_a = ln sum_{b!=a} exp(2 sim_ab);  pos_a = sim[a, a+-N]
    loss = mean(lse_a - 2 pos_a)

Triangle sharding, core-uniform: on the 64x64 grid of 128x128 sim tiles,
each unordered tile pair is computed once:
  * super-diagonal 8x8-tile block I: internal upper triangle, computed by
    core I against its own row tiles ("zrows" slots 0-7, rhs = itself).
  * suffix strips: row tile 8*si + c (core c, "zrows" slot 8+si) x columns
    [8*(si+1), 64): the column START is uniform across cores; core identity
    lives only in the input data.  260 exp-tiles per core, balanced.
Row sums come from ACTIVATE-exp accumulators; column sums (mirrored lower
triangle) accumulate in bf16 E_acc/E_int via DVE/GpSimd adds; the host
finishes the reduction and the final ln/mean ("all-reduce").

Engine plan:
  * transposes on the PE (is_transpose matmul, bf16 -> bf16 PSUM), eight
    128x128 tile-halves per PSUM bank; GpSimd copies PSUM -> SBUF as fp8.
    (The DMA XBAR path measures ~82 GB/s and serializes against loads --
    too slow for the 4.5 MB of transposed reps.)
  * main matmuls in fp8e4 DoubleRow (K=256 folded into one pass).
  * exp in <=1536-wide ACTIVATEs on ScalarE (PSUM: 2x3 sim banks + 2
    transpose banks), rowsum via accum_out.
  * element-wise prep (sumsq, invn scale, colsum adds) on DVE; GpSimd adds
    take a minority share.  Emission is software-pipelined per engine so
    in-order queues never park a prep behind main-loop consumers.
"""

import sys

for _p in ("/opt/trn_rl_repo",):
    if _p not in sys.path:
        sys.path.insert(0, _p)

import numpy as np
from contextlib import ExitStack

import concourse.bass as bass
import concourse.tile as tile
from concourse import mybir
from concourse.masks import make_identity
from concourse.vector_clock import ScopedClock as _ScopedClock


def _patched_drain_and_barrier(self, tick_clock, wait_clock):
    """Tile's closing drain carries one sem-wait per DMA lane used, but this
    walrus build only accepts a single sync wait on a Drain (CTRL-NO)
    lowering ("Too many sync wait commands").  Split the waits across a
    chain of drains (sequential on SP, so semantics are unchanged)."""
    nc = self.nc
    drain_inst = nc.sync.drain()
    wait_clock.add_sem_waits(
        drain_inst.ins, _ScopedClock({None: tick_clock.global_clock})
    )
    si = drain_inst.ins.sync_info
    if si is not None:
        waits = list(si.on_wait or [])
        if len(waits) > 1:
            import bass_rust as _br

            si.on_wait = waits[:1]
            for w in waits[1:]:
                d2 = nc.sync.drain()
                d2.ins.sync_info = _br.SyncInfo(on_wait=[w], on_update=[])
    nc.all_engine_barrier()
    assert self.sems is not None
    popped = nc._tile_sem_poison_stack.pop()
    assert popped is self._sem_poison
    nc.clear_and_free_semaphores(list(self.sems.allocated().values()))
    nc.all_engine_barrier()


tile.TileContext._drain_and_barrier = _patched_drain_and_barrier

_orig_lower_ordered = tile.TileContext._lower_ordered_insts


def _split_multiwaits_and_lower(self, ordered):
    """Same walrus limitation as above, for scheduled compute/DMA
    instructions: hoist all but one sync wait onto single-wait NoOps that
    precede the instruction on its own engine."""
    nc = self.nc
    for insts in ordered.values():
        if not any(
            inst.sync_info is not None and len(inst.sync_info.on_wait or []) > 1
            for inst in insts
        ):
            continue
        out = []
        for inst in insts:
            si = inst.sync_info
            waits = list(si.on_wait) if si is not None and si.on_wait else []
            if len(waits) > 1 and getattr(inst, "engine", None) is not None:
                for w in waits[:-1]:
                    out.append(
                        mybir.InstNoOp(
                            name=nc.get_next_instruction_name(),
                            sync_info=mybir.SyncInfo(on_wait=[w], on_update=[]),
                            bass_nofuse=True,
                            engine=inst.engine,
                        )
                    )
                si.on_wait = waits[-1:]
            out.append(inst)
        insts[:] = out
    return _orig_lower_ordered(self, ordered)


tile.TileContext._lower_ordered_insts = _split_multiwaits_and_lower

N_CORES = 8
N_FULL = 4096
D_FULL = 256

f32 = mybir.dt.float32
f16 = mybir.dt.float16
bf16 = mybir.dt.bfloat16
fp8 = mybir.dt.float8e4
ALU = mybir.AluOpType
AF = mybir.ActivationFunctionType
DR = mybir.MatmulPerfMode.DoubleRow

# descending 16-tile chunks of the global column range [8, 64)
CHUNKS = [(48, 16), (32, 16), (16, 16), (8, 8)]
GW = 12  # max ACT group width in tiles (PSUM sim tile = 1536 f32 = 3 banks)


def plan_groups():
    """Emission-ordered sim/exp groups shared by build_bass and assemble.

    Returns a list of dicts: owner ('int', a) or ('strip', si); lhs slot in
    zrows; t0/w column tiles (zrows-local for 'int', global otherwise);
    sp_slot; colsum first-touch flag; acc_t0 (first colsum tile or None).
    """
    groups = []
    slot = 0

    def add(owner, lhs_slot, t0, w, acc_t0, ft):
        nonlocal slot
        groups.append(dict(owner=owner, lhs=lhs_slot, t0=t0, w=w,
                           acc_t0=acc_t0, ft=ft, slot=slot))
        slot += 1

    # internal triangle of the super-diagonal block (zrows slots 0-7)
    for a in range(8):
        add(("int", a), a, a, 8 - a, a + 1, a == 0)
    # suffix strips, chunk-streamed, split to <= GW tiles
    for step, (c0, nt) in enumerate(CHUNKS):
        for si in range(7):
            t0 = max(8 * (si + 1), c0)
            t1 = c0 + nt
            p0 = t0
            while p0 < t1:
                w = min(GW, t1 - p0)
                add(("strip", si), 8 + si, p0, w, p0, si == 0)
                p0 += w
    return groups


GROUPS = plan_groups()
N_SLOTS = len(GROUPS)


def build_bass(N=N_FULL, D=D_FULL, n_cores=N_CORES):
    n2 = 2 * N
    TF = n2 // 128            # 64 global tiles
    KH = D // 128             # 2 contraction halves
    NR = 16                   # zrows tiles: 8 super-diag + 8 strip rows
    assert D == 256 and TF == 64

    nc = bass.Bass()
    z_i = nc.declare_dram_parameter("z_i", [N, D], f32, isOutput=False)
    z_j = nc.declare_dram_parameter("z_j", [N, D], f32, isOutput=False)
    zr = nc.declare_dram_parameter("zrows", [NR * 128, D], f32, isOutput=False)
    eacc_out = nc.declare_dram_parameter("eacc", [128, n2], bf16, isOutput=True)
    eint_out = nc.declare_dram_parameter("eint", [128, 1024], bf16, isOutput=True)
    sp_out = nc.declare_dram_parameter("spart", [128, N_SLOTS], f32, isOutput=True)
    pos_out = nc.declare_dram_parameter("posr", [128, 8], f32, isOutput=True)

    with ExitStack() as ctx:
        tc = ctx.enter_context(tile.TileContext(nc))
        big = ctx.enter_context(tc.tile_pool(name="big", bufs=1))
        zfs = ctx.enter_context(tc.tile_pool(name="zfs", bufs=3))
        zns = ctx.enter_context(tc.tile_pool(name="zns", bufs=2))
        sqs = ctx.enter_context(tc.tile_pool(name="sqs", bufs=2))
        escr = ctx.enter_context(tc.tile_pool(name="escr", bufs=3))
        pmm = ctx.enter_context(tc.tile_pool(name="pmm", bufs=2, space="PSUM"))
        pmt = ctx.enter_context(tc.tile_pool(name="pmt", bufs=2, space="PSUM"))

        ident = big.tile([128, 128], bf16)
        make_identity(nc, ident)

        znT8 = big.tile([128, KH, n2], fp8)       # global transposed fp8
        ssq = big.tile([128, TF], f16)
        lnss = big.tile([128, TF], f32)
        invn = big.tile([128, TF], f32)

        znr = big.tile([128, NR, D], bf16)        # zrows normalized (pos)
        zrT8 = big.tile([128, KH, NR * 128], fp8)
        ssqr = big.tile([128, NR], f16)
        lnssr = big.tile([128, NR], f32)
        invnr = big.tile([128, NR], f32)

        E_acc = big.tile([128, n2], bf16)         # strip colsum partials
        E_int = big.tile([128, 1024], bf16)       # super-diag colsum partials
        Spart = big.tile([128, N_SLOTS], f32)     # rowsum accum slots
        posr = big.tile([128, 8], f32)

        zi_r = z_i[:, :].rearrange("(t p) d -> p t d", p=128)
        zj_r = z_j[:, :].rearrange("(t p) d -> p t d", p=128)

        # ---- all loads issued up front; SWDGE queue + pool WAR deps give
        # natural flow control (zfs bufs=3)
        zf_tiles = []
        zf_r = zfs.tile([128, 16, D], bf16, tag="zf")
        nc.gpsimd.dma_start(out=zf_r[:, :, :],
                            in_=zr[:, :].rearrange("(t p) d -> p t d", p=128))
        for (c0, nt) in CHUNKS:
            src = (zj_r[:, c0 - 32 : c0 - 32 + nt, :] if c0 >= 32
                   else zi_r[:, c0 : c0 + nt, :])
            zf = zfs.tile([128, 16, D], bf16, tag="zf")
            nc.gpsimd.dma_start(out=zf[:, :nt, :], in_=src)
            zf_tiles.append(zf)

        def prep_dve(zf, nt, dst_zn, d_ssq, d_lnss, d_invn, qoff):
            """sumsq -> invn -> normalize (DVE + ScalarE only)."""
            sq = sqs.tile([128, 16, D], bf16, tag="sq")
            q = slice(qoff, qoff + nt)
            nc.vector.tensor_mul(out=sq[:, :nt, :], in0=zf[:, :nt, :],
                                 in1=zf[:, :nt, :])
            with nc.allow_low_precision("fp16 sumsq: |z|^2~256, rel 5e-4"):
                nc.vector.reduce_sum(out=d_ssq[:, q], in_=sq[:, :nt, :],
                                     axis=mybir.AxisListType.X)
            nc.scalar.activation(out=d_lnss[:, q], in_=d_ssq[:, q], func=AF.Ln)
            nc.scalar.activation(out=d_invn[:, q], in_=d_lnss[:, q],
                                 func=AF.Exp, scale=-0.5)
            zn = dst_zn if dst_zn is not None else zns.tile(
                [128, 16, D], bf16, tag="zn")
            for k in range(nt):
                nc.vector.tensor_scalar_mul(
                    out=zn[:, k, :], in0=zf[:, k, :],
                    scalar1=d_invn[:, qoff + k : qoff + k + 1],
                )
            return zn

        copy_flip = [0]

        def prep_pe_batches(zn, nt, dst_T8, toff):
            """PE tile-half transposes -> PSUM bf16 -> fp8 copies, as a list
            of thunks (one per PSUM bank) so they can be interleaved with
            sim groups on the PE queue.  GpSimd cannot read PSUM; the copies
            alternate DVE/ScalarE."""
            def batch(h, g0, gn):
                def run():
                    pt = pmt.tile([128, 1024], bf16, tag="pt")
                    for k in range(gn):
                        nc.tensor.matmul(
                            out=pt[:, k * 128 : (k + 1) * 128],
                            lhsT=zn[:, g0 + k, h * 128 : (h + 1) * 128],
                            rhs=ident, is_transpose=True,
                        )
                    dst = dst_T8[:, h,
                                 (toff + g0) * 128 : (toff + g0 + gn) * 128]
                    if copy_flip[0] % 3 == 2:
                        nc.scalar.copy(out=dst, in_=pt[:, : gn * 128])
                    else:
                        nc.vector.tensor_copy(out=dst, in_=pt[:, : gn * 128])
                    copy_flip[0] += 1
                return run
            return [batch(h, g0, min(8, nt - g0))
                    for h in range(KH) for g0 in range(0, nt, 8)]

        def prep_pe(zn, nt, dst_T8, toff):
            for b in prep_pe_batches(zn, nt, dst_T8, toff):
                b()

        def sim_exp_group(g):
            """MMs -> exp(+rowsum accum) -> colsum accumulate for one group."""
            t0, w, slot = g["t0"], g["w"], g["slot"]
            internal = g["owner"][0] == "int"
            lhs = zrT8
            rhs = zrT8 if internal else znT8
            cols = w * 128
            ps = pmm.tile([128, GW * 128], f32, tag="ps")
            for j0 in range(0, w, 4):
                wj = min(4, w - j0) * 128
                c0 = (t0 + j0) * 128
                nc.tensor.matmul(
                    out=ps[:, j0 * 128 : j0 * 128 + wj],
                    lhsT=lhs[:, :, g["lhs"] * 128 : (g["lhs"] + 1) * 128],
                    rhs=rhs[:, :, c0 : c0 + wj],
                    start=True, stop=True, perf_mode=DR,
                )
            dst_buf = E_int if internal else E_acc
            if g["ft"]:
                # first touch of these E columns: the exp output IS the
                # colsum partial -- write it there directly, no copy.
                # (E columns left of acc_t0 get written too; the host
                # ignores them for 'int' and they are never first-touch
                # overlaps for strips since si=0 spans every range.)
                e = dst_buf[:, t0 * 128 : (t0 + w) * 128]
            else:
                e_t = escr.tile([128, GW * 128], bf16, tag="e")
                e = e_t[:, :cols]
            nc.scalar.activation(
                out=e, in_=ps[:, :cols], func=AF.Exp, scale=2.0,
                accum_out=Spart[:, slot : slot + 1],
            )
            if g["ft"]:
                return
            a, b = g["acc_t0"], t0 + w
            if a >= b:
                return
            d = dst_buf[:, a * 128 : b * 128]
            s = e[:, (a - t0) * 128 : (b - t0) * 128]
            eng = nc.gpsimd if slot % 2 == 1 else nc.vector
            eng.tensor_tensor(out=d, in0=d, in1=s, op=ALU.add)

        # group emission bookkeeping
        int_groups = [g for g in GROUPS if g["owner"][0] == "int"]
        step_groups = [[] for _ in CHUNKS]
        for g in GROUPS:
            if g["owner"][0] == "strip":
                for step, (c0, nt) in enumerate(CHUNKS):
                    if c0 <= g["t0"] < c0 + nt:
                        step_groups[step].append(g)

        # ---- software-pipelined emission; transpose batches interleave
        # with sim groups so the PE queue never parks ready work behind a
        # group whose PSUM tile is still draining ----
        znr_t = prep_dve(zf_r, 16, znr, ssqr, lnssr, invnr, 0)
        prep_pe(znr_t, 16, zrT8, 0)
        zn0 = prep_dve(zf_tiles[0], CHUNKS[0][1], None, ssq, lnss, invn,
                       CHUNKS[0][0])
        pending = prep_pe_batches(zn0, CHUNKS[0][1], znT8, CHUNKS[0][0])
        for g in int_groups:
            sim_exp_group(g)
            if pending:
                pending.pop(0)()
        for step in range(len(CHUNKS)):
            while pending:
                pending.pop(0)()
            if step + 1 < len(CHUNKS):
                c0, nt = CHUNKS[step + 1]
                zn1 = prep_dve(zf_tiles[step + 1], nt, None, ssq, lnss, invn,
                               c0)
                pending = prep_pe_batches(zn1, nt, znT8, c0)
            for g in step_groups[step]:
                sim_exp_group(g)
                if pending:
                    pending.pop(0)()

        # ---- positive pairs: zrows strip slots si=0..3 vs si+4 ----
        pmul = big.tile([128, 4, D], bf16)
        nc.vector.tensor_mul(out=pmul[:, :, :], in0=znr[:, 8:12, :],
                             in1=znr[:, 12:16, :])
        nc.vector.reduce_sum(out=posr[:, 0:4], in_=pmul[:, :, :],
                             axis=mybir.AxisListType.X)

        nc.sync.dma_start(out=eacc_out[:, :], in_=E_acc)
        nc.sync.dma_start(out=eint_out[:, :], in_=E_int)
        nc.sync.dma_start(out=sp_out[:, :], in_=Spart)
        nc.sync.dma_start(out=pos_out[:, :], in_=posr)

    return nc


_NC_CACHE = {}


def _get_nc(N=N_FULL, D=D_FULL):
    key = (N, D)
    if key not in _NC_CACHE:
        _NC_CACHE[key] = build_bass(N, D)
    return _NC_CACHE[key]


def make_in_maps(z_i, z_j, n_cores=N_CORES):
    z_i = np.ascontiguousarray(z_i, dtype=np.float32)
    z_j = np.ascontiguousarray(z_j, dtype=np.float32)
    reps = np.concatenate([z_i, z_j], axis=0)
    maps = []
    for c in range(n_cores):
        rows = [reps[(8 * c + a) * 128 : (8 * c + a + 1) * 128] for a in range(8)]
        rows += [reps[(8 * si + c) * 128 : (8 * si + c + 1) * 128]
                 for si in range(8)]
        maps.append({
            "z_i": z_i,
            "z_j": z_j,
            "zrows": np.ascontiguousarray(np.concatenate(rows, axis=0)),
        })
    return maps


def assemble(results, N=N_FULL):
    """Host-side gather + reduction + final ln/mean ("all-reduce")."""
    n2 = 2 * N
    S = np.zeros(n2, dtype=np.float64)
    pos = np.zeros(N, dtype=np.float64)
    for c in range(N_CORES):
        r0 = results[c]
        sp = np.asarray(r0["spart"], dtype=np.float64)
        ea = np.asarray(r0["eacc"], dtype=np.float32)
        ei = np.asarray(r0["eint"], dtype=np.float32)
        for g in GROUPS:
            kind, idx = g["owner"]
            r = 8 * c + idx if kind == "int" else 8 * idx + c
            S[r * 128 : (r + 1) * 128] += sp[:, g["slot"]]
        # super-diag colsums: E_int col b*128+q -> row (8c+b)*128+q
        eint_sum = ei[:, 128:].astype(np.float64).sum(axis=0)
        S[8 * c * 128 + 128 : 8 * c * 128 + 1024] += eint_sum
        # strip colsums: global columns >= 1024
        S[1024:] += ea[:, 1024:].astype(np.float64).sum(axis=0)
        # pos: strip slot si covers tile 8si+c (si=0..3)
        pr = np.asarray(r0["posr"], dtype=np.float64)
        for si in range(4):
            t = 8 * si + c
            pos[t * 128 : (t + 1) * 128] = pr[:, si]
    S -= np.exp(2.0)  # self-similarity exp(2*|zn_a|^2)
    lse = np.log(S)
    loss = np.mean(lse - 2.0 * np.concatenate([pos, pos]))
    return np.float32(loss)


def _run(z_i, z_j, trace=False, tmpdir=None, **spmd_kwargs):
    from concourse.bass_utils import run_bass_kernel_spmd

    N, D = z_i.shape
    nc = _get_nc(N, D)
    in_maps = make_in_maps(z_i, z_j)
    out = run_bass_kernel_spmd(
        nc, in_maps, list(range(N_CORES)), trace=trace, tmpdir=tmpdir,
        **spmd_kwargs
    )
    return assemble(out.results, N), out


def kernel(z_i, z_j):
    loss, _ = _run(np.asarray(z_i), np.asarray(z_j))
    return loss


if __name__ == "__main__":
    rng = np.random.default_rng(0)
    z_i = rng.standard_normal((N_FULL, D_FULL), dtype=np.float32)
    z_j = rng.standard_normal((N_FULL, D_FULL), dtype=np.float32)
    print(kernel(z_i, z_j))
